# revision 1
# baseline (speedup 1.0000x reference)
"""BlockDCTSandwich Trainium2 kernel.

The whole op (blockify -> 8x8 DCT -> zigzag gather -> Linear(64,64) -> IDCT
-> deblockify) is a single fused 64x64 linear map per 8x8 block:
    out_vec = M @ x_vec + c,  M = kron(D^T,D^T) @ W @ G @ kron(D,D),
    c = kron(D^T,D^T) @ bias
(everything is linear; G is the gather matrix for the zigzag reorder).

On-chip dataflow per [128, 512] image tile (data-parallel over batch, one
batch element per NeuronCore):
  DMA loads rows with partition p = n*16 + hb   (h = 8*hb + n)
  T1 (DVE 32x32 stream transpose, strided view)  X -> Y
  T2 (DVE stream transpose, flat view)           Y -> Z
     Z[p = n*16 + m*2 + wb5, hb*32 + wbl] = x[8hb+n, (wb5*32+wbl)*8 + m]
  MM: one stationary 128x128 weight blkdiag-encodes M for the two block
     columns (wb5 = 0/1) stacked per partition -> PSUM
  ACT copy PSUM->SBUF, then the two inverse transposes mirror T2/T1,
  DMA stores rows back.

Self-contained: hardcodes shapes x=(8,16,512,512) f32, W=(64,64), bias=(64,).
"""

import sys

import numpy as np

if "/opt/trn_rl_repo" not in sys.path:
    sys.path.insert(0, "/opt/trn_rl_repo")

_B = 8
_NCORES = 8


def _dct_matrix(b):
    n = np.arange(b)
    k = n[:, None]
    Dm = np.sqrt(2.0 / b) * np.cos(np.pi * (2 * n[None, :] + 1) * k / (2 * b))
    Dm[0] *= 1.0 / np.sqrt(2.0)
    return Dm


def _build_idx(b):
    def to_key(x):
        s = x[0] + x[1]
        o = b * b * s
        if s % 2 == 1:
            o += x[0]
        else:
            o -= x[0]
        return o

    coords = sorted(([i, j] for i in range(b) for j in range(b)), key=to_key)
    arr = np.array(coords).reshape(b, b, 2)
    return (np.arange(b)[None, :] * arr[..., 0] + arr[..., 1]).reshape(-1)


def _consts(W, bias):
    """Fused 64x64 map M, its 128x128 stationary lhsT, and bias vector c.

    lhsT row (input) encoding comes from the PE-transpose forward path:
        pi = m0*64 + wb5*32 + n*4 + m2*2 + m1   (m = m2*4 + m1*2 + m0)
    lhsT col (output) encoding is the DVE inverse-dance layout:
        po = n*16 + m*2 + s
    """
    D = _dct_matrix(_B)
    idx = _build_idx(_B)
    G = np.zeros((64, 64))
    G[np.arange(64), idx] = 1.0
    M = np.kron(D.T, D.T) @ W.astype(np.float64) @ G @ np.kron(D, D)
    c = np.kron(D.T, D.T) @ bias.astype(np.float64)
    LT = np.zeros((128, 128), np.float64)
    for a in range(128):
        m0, s_i = a >> 6, (a >> 5) & 1
        n_i = (a >> 2) & 7
        m_i = ((a >> 1) & 1) * 4 + (a & 1) * 2 + m0
        for b_ in range(128):
            if s_i == (b_ & 1):
                LT[a, b_] = M[8 * (b_ >> 4) + ((b_ >> 1) & 7), 8 * n_i + m_i]
    return LT.astype(np.float32), c


_NC_CACHE = {}


def _build_nc():
    if "nc" in _NC_CACHE:
        return _NC_CACHE["nc"]
    import concourse.bass as bass
    import concourse.mybir as mybir
    from concourse import bacc
    from concourse.tile import TileContext

    f32 = mybir.dt.float32
    ds = bass.ds

    nc = bacc.Bacc("TRN2", target_bir_lowering=False, debug=False,
                   num_devices=_NCORES)
    xin = nc.dram_tensor("xin", [8192, 512], f32, kind="ExternalInput")
    ltw = nc.dram_tensor("ltw", [128, 128], f32, kind="ExternalInput")
    idw = nc.dram_tensor("idw", [128, 128], f32, kind="ExternalInput")
    yout = nc.dram_tensor("yout", [8192, 512], f32, kind="ExternalOutput")

    xin_ap = xin.ap()
    yout_ap = yout.ap()

    with TileContext(nc) as tc:
        with (
            tc.tile_pool(name="wp", bufs=1) as wp,
            tc.tile_pool(name="io", bufs=4) as iop,
            tc.tile_pool(name="wk", bufs=6) as wk,
            tc.tile_pool(name="psp", bufs=4, space="PSUM") as psp,
        ):
            lt_sb = wp.tile([128, 128], f32)
            nc.sync.dma_start(out=lt_sb[:, :], in_=ltw.ap())
            id_sb = wp.tile([128, 128], f32, tag="id_sb")
            nc.sync.dma_start(out=id_sb[:, :], in_=idw.ap())

            def x_view(ap, w5):
                # X-layout half, view (m, wl): strides 1, 8; offset w5*256
                return ap.rearrange(
                    "p (w5 wl m) -> p w5 wl m", w5=2, wl=32, m=8
                )[:, w5].transpose([0, 2, 1])

            def y_view(ap, w5):
                # Y-layout half, view (m, n0, hb): strides 2, 16, 32; offset w5
                return ap.rearrange(
                    "p (hb n0 m w5) -> p hb n0 m w5", hb=16, n0=2, m=8, w5=2
                )[:, :, :, :, w5].transpose([0, 3, 2, 1])

            for TB in range(16):  # one channel (512 rows = 4 tiles) per TB
                XB = iop.tile([128, 2048], f32, tag="XB")
                # natural load: partition = local row r = 8*hb + n
                nc.sync.dma_start(
                    out=XB[:, :],
                    in_=xin_ap[ds(TB * 512, 512), :]
                    .rearrange("(t4 r) w -> t4 r w", t4=4, r=128)
                    .transpose([1, 0, 2]),
                )
                OXB = iop.tile([128, 2048], f32, tag="OXB")
                for t4 in range(4):
                    Xs = XB[:, ds(t4 * 512, 512)]
                    # forward blockify on PE: 4 transpose-matmuls, chunk
                    # c = m2*2+m1; lhsT cols enumerate (m0, wb)
                    psT = psp.tile([128, 512], f32, tag="psT")
                    xs4 = Xs.rearrange("p (wb m) -> p wb m", wb=64, m=8)
                    for cc in range(4):
                        for m0 in range(2):
                            nc.tensor.matmul(
                                psT[ds(64 * m0, 64), ds(128 * cc, 128)],
                                xs4[:, :, 2 * cc + m0],
                                id_sb[:, :], start=True, stop=True,
                                tile_position=(0, 64 * m0),
                            )
                    # ACT copy PSUM->SBUF into V layout f = hb*32 + n*4 + c
                    V = wk.tile([128, 512], f32, tag="V")
                    nc.scalar.copy(
                        V[:, :].rearrange("p (hb n c) -> p hb n c",
                                          hb=16, n=8, c=4).transpose([0, 3, 1, 2]),
                        psT[:, :].rearrange("p (c hb n) -> p c hb n",
                                            c=4, hb=16, n=8),
                    )
                    # DVE 32-block transpose -> Z2 rows = lhsT input encoding
                    Z = wk.tile([128, 512], f32, tag="Z")
                    nc.vector.transpose(out=Z[:, :], in_=V[:, :])
                    ps = psp.tile([128, 512], f32, tag="ps")
                    nc.tensor.matmul(ps[:, :], lt_sb[:, :], Z[:, :],
                                     start=True, stop=True)
                    # inverse dance (DVE), T2' reads PSUM directly
                    OY = wk.tile([128, 512], f32, tag="OY")
                    nc.vector.transpose(out=OY[:, :], in_=ps[:, :])
                    for w5 in range(2):
                        nc.vector.transpose(
                            out=x_view(OXB[:, ds(t4 * 512, 512)], w5),
                            in_=y_view(OY[:, :], w5),
                        )
                for t4 in range(4):
                    odst = (
                        yout_ap[ds(TB * 512 + t4 * 128, 128), :]
                        .rearrange("(hb n) w -> hb n w", hb=16, n=8)
                        .transpose([1, 0, 2])
                    )
                    nc.scalar.dma_start(out=odst, in_=OXB[:, ds(t4 * 512, 512)])

    nc.finalize()
    _NC_CACHE["nc"] = nc
    return nc


def run(x, W, bias, trace=False):
    from concourse.bass_utils import run_bass_kernel_spmd

    x = np.ascontiguousarray(np.asarray(x, dtype=np.float32))
    W = np.asarray(W, dtype=np.float32)
    bias = np.asarray(bias, dtype=np.float32)
    assert x.shape == (8, 16, 512, 512), x.shape

    LT, c = _consts(W, bias)
    nc = _build_nc()
    ident = np.eye(128, dtype=np.float32)
    in_maps = [
        {"xin": np.ascontiguousarray(x[i].reshape(8192, 512)), "ltw": LT,
         "idw": ident}
        for i in range(_NCORES)
    ]
    res = run_bass_kernel_spmd(nc, in_maps, core_ids=list(range(_NCORES)),
                               trace=trace)
    out = np.stack(
        [res.results[i]["yout"].reshape(16, 512, 512) for i in range(_NCORES)]
    )
    if np.any(c):
        cimg = np.tile(c.reshape(8, 8), (64, 64)).astype(np.float32)
        out = out + cimg[None, None]
    return out.astype(np.float32), res


def kernel(x, W, bias):
    out, _ = run(x, W, bias, trace=False)
    return out



# revision 15
# speedup vs baseline: 2.1339x; 2.1339x over previous
"""BlockDCTSandwich Trainium2 kernel.

The whole op (blockify -> 8x8 DCT -> zigzag gather -> Linear(64,64) -> IDCT
-> deblockify) is a single fused 64x64 linear map per 8x8 block:
    out_vec = M @ x_vec + c,  M = kron(D^T,D^T) @ W @ G @ kron(D,D),
    c = kron(D^T,D^T) @ bias

Data-parallel over batch: one batch element (16 channels, 16.78 MB) per
NeuronCore. Per [128, 512] image tile (rows r = 8*hb + n, cols w = 8*wb + m,
wb = s*32 + sw_lo, m = 2*cc + m0):

  load   natural rows (partition = r), SWDGE cast f32 -> bf16
  fwd    8 PE transposes (bf16, tile_position packs m0) -> psT PSUM bf16
            psT[m0*64+wb, cc*128 + r] = x[r, wb*8 + cc*2 + m0]
  Z      1 DVE stream-transpose (strided PSUM view)     -> Z SBUF bf16
            Z[pi, hb*32 + sw] = x[hb*8+n, (s*32+sw)*8 + m]
            pi = m0*64 + s*32 + n*4 + cc
  mm     1 bf16 matmul, stationary LT encodes M blockdiag over s -> ps PSUM
  T_a    1 DVE stream-transpose (strided SBUF view)     -> OYs SBUF f32
  inv    4 PE transposes (f32)                          -> psO PSUM f32
  evac   1 ACT copy (free-dim shuffle) -> OXB natural   -> store f32

Self-contained: hardcodes shapes x=(8,16,512,512) f32, W=(64,64), bias=(64,).
"""

import sys

import numpy as np

if "/opt/trn_rl_repo" not in sys.path:
    sys.path.insert(0, "/opt/trn_rl_repo")

_B = 8
_NCORES = 8


def _dct_matrix(b):
    n = np.arange(b)
    k = n[:, None]
    Dm = np.sqrt(2.0 / b) * np.cos(np.pi * (2 * n[None, :] + 1) * k / (2 * b))
    Dm[0] *= 1.0 / np.sqrt(2.0)
    return Dm


def _build_idx(b):
    def to_key(x):
        s = x[0] + x[1]
        o = b * b * s
        if s % 2 == 1:
            o += x[0]
        else:
            o -= x[0]
        return o

    coords = sorted(([i, j] for i in range(b) for j in range(b)), key=to_key)
    arr = np.array(coords).reshape(b, b, 2)
    return (np.arange(b)[None, :] * arr[..., 0] + arr[..., 1]).reshape(-1)


def _consts(W, bias):
    """Fused 64x64 map M as a 128x128 stationary lhsT (blockdiag over the
    column-half parity s), plus the bias image constant c.

    Partition encoding on both matmul sides: a = m0*64 + s*32 + n*4 + cc,
    with block-local coords (n, m), m = 2*cc + m0.
    """
    D = _dct_matrix(_B)
    idx = _build_idx(_B)
    G = np.zeros((64, 64))
    G[np.arange(64), idx] = 1.0
    M = np.kron(D.T, D.T) @ W.astype(np.float64) @ G @ np.kron(D, D)
    c = np.kron(D.T, D.T) @ bias.astype(np.float64)

    enc = np.arange(128)
    loc = 8 * ((enc >> 2) & 7) + 2 * (enc & 3) + (enc >> 6)
    spar = (enc >> 5) & 1
    LT = M[np.ix_(loc, loc)].T * (spar[:, None] == spar[None, :])
    return LT.astype(np.float32), c


_NC_CACHE = {}


def _build_nc():
    if "nc" in _NC_CACHE:
        return _NC_CACHE["nc"]
    import concourse.bass as bass
    import concourse.mybir as mybir
    from concourse import bacc
    from concourse.tile import TileContext

    f32 = mybir.dt.float32
    bf16 = mybir.dt.bfloat16
    ds = bass.ds

    nc = bacc.Bacc("TRN2", target_bir_lowering=False, debug=False,
                   num_devices=_NCORES)
    xin = nc.dram_tensor("xin", [8192, 512], f32, kind="ExternalInput")
    ltw = nc.dram_tensor("ltw", [128, 128], bf16, kind="ExternalInput")
    idw = nc.dram_tensor("idw", [128, 128], bf16, kind="ExternalInput")
    idwf = nc.dram_tensor("idwf", [128, 128], f32, kind="ExternalInput")
    yout = nc.dram_tensor("yout", [8192, 512], f32, kind="ExternalOutput")

    xin_ap = xin.ap()
    yout_ap = yout.ap()

    with TileContext(nc) as tc:
        with (
            tc.tile_pool(name="wp", bufs=1) as wp,
            tc.tile_pool(name="io", bufs=3) as iop,
            tc.tile_pool(name="wk", bufs=4) as wk,
            tc.tile_pool(name="psp", bufs=2, space="PSUM") as psp,
        ):
            lt_sb = wp.tile([128, 128], bf16)
            nc.sync.dma_start(out=lt_sb[:, :], in_=ltw.ap())
            id_sb = wp.tile([128, 128], bf16, tag="id_sb")
            nc.sync.dma_start(out=id_sb[:, :], in_=idw.ap())
            idf_sb = wp.tile([128, 128], f32, tag="idf_sb")
            nc.sync.dma_start(out=idf_sb[:, :], in_=idwf.ap())

            # Software-pipelined flat loop over 32 tile-PAIRS (each pair =
            # two [128, 512] tiles, 1024-wide DVE transposes to amortize the
            # PSUM-access init). Steady-state issue per iter p:
            #   mm_p | Z_{p+1} | inv_{p-1} | T_a_p | fwdT_{p+2} | evac/store
            # DVE (bottleneck) order [Z_{p+1}, T_a_p] never stalls: mm_p
            # (427ns) completes inside Z_{p+1} (1192ns).
            NP = 32
            XBs, psTs, Zs, pss, OYss, psOs = {}, {}, {}, {}, {}, {}

            def load(b):
                if b > 15 or b in XBs:
                    return
                XB = iop.tile([128, 2048], bf16, tag="XB")
                nc.gpsimd.dma_start(
                    out=XB[:, :],
                    in_=xin_ap[ds(b * 512, 512), :]
                    .rearrange("(t4 r) w -> t4 r w", t4=4, r=128)
                    .transpose([1, 0, 2]),
                )
                XBs[b] = XB

            def fwd_psT(p):
                if p >= NP:
                    return
                b = p // 2
                load(b)
                psT = psp.tile([128, 1024], bf16, tag="psT")
                for t in range(2):
                    Xs = XBs[b][:, ds(((p % 2) * 2 + t) * 512, 512)]
                    xv = Xs.rearrange("p (wb m) -> p wb m", wb=64, m=8)
                    for m in range(8):
                        cc, m0 = m >> 1, m & 1
                        nc.tensor.transpose(
                            psT[ds(64 * m0, 64),
                                ds(cc * 256 + t * 128, 128)],
                            in_=xv[:, :, m],
                            identity=id_sb[:, :],
                            tile_position=(0, 64 * m0),
                        )
                psTs[p] = psT

            def z_step(p, split=False):
                if p >= NP:
                    return
                Z = wk.tile([128, 1024], bf16, tag="Z")
                psT = psTs.pop(p)
                if split:  # two halves so DVE starts before full psT ready
                    for t in range(2):
                        nc.vector.transpose(
                            out=Z[:, ds(t * 512, 512)],
                            in_=psT[:, :].rearrange(
                                "p (cc t hb n) -> p t hb n cc",
                                cc=4, t=2, hb=16, n=8)[:, t],
                        )
                else:
                    nc.vector.transpose(
                        out=Z[:, :],
                        in_=psT[:, :].rearrange(
                            "p (cc thb n) -> p thb n cc", cc=4, thb=32, n=8),
                    )
                Zs[p] = Z

            def mm_step(p):
                ps = psp.tile([128, 1024], f32, tag="ps")
                Z = Zs.pop(p)
                for t in range(2):
                    nc.tensor.matmul(ps[:, ds(t * 512, 512)], lt_sb[:, :],
                                     Z[:, ds(t * 512, 512)],
                                     start=True, stop=True)
                pss[p] = ps

            def ta_step(p):
                OYs = wk.tile([128, 1024], f32, tag="OYs")
                nc.vector.transpose(
                    out=OYs[:, :].rearrange("p (cc thb n) -> p thb n cc",
                                            cc=4, thb=32, n=8),
                    in_=pss.pop(p)[:, :],
                )
                OYss[p] = OYs

            def inv_step(p):
                if p < 0:
                    return
                OYs = OYss.pop(p)
                tiles = []
                for t in range(2):
                    psO = psp.tile([128, 512], f32, tag="psO")
                    for cc in range(4):
                        nc.tensor.transpose(
                            psO[:, ds(cc * 128, 128)],
                            in_=OYs[:, ds(cc * 256 + t * 128, 128)],
                            identity=idf_sb[:, :],
                        )
                    tiles.append(psO)
                psOs[p] = tiles

            def evac_store(p):
                if p < 0:
                    return
                OXP = iop.tile([128, 1024], f32, tag="OXP")
                for t, psO in enumerate(psOs.pop(p)):
                    nc.scalar.copy(
                        OXP[:, ds(t * 512, 512)].rearrange(
                            "p (sw cc m0) -> p cc m0 sw", sw=64, cc=4, m0=2),
                        psO[:, :].rearrange(
                            "p (cc m0 sw) -> p cc m0 sw", cc=4, m0=2, sw=64),
                    )
                    nc.sync.dma_start(
                        out=yout_ap[ds(p * 256 + t * 128, 128), :],
                        in_=OXP[:, ds(t * 512, 512)],
                    )

            # prologue: first channel block loads as two halves so fwdT_0
            # (which only needs the first 1024 cols) starts ~1.5us earlier
            XB0 = iop.tile([128, 2048], bf16, tag="XB")
            for h in range(2):
                nc.gpsimd.dma_start(
                    out=XB0[:, ds(h * 1024, 1024)],
                    in_=xin_ap[ds(h * 256, 256), :]
                    .rearrange("(t2 r) w -> t2 r w", t2=2, r=128)
                    .transpose([1, 0, 2]),
                )
            XBs[0] = XB0
            load(1)
            fwd_psT(0)
            z_step(0, split=True)
            fwd_psT(1)

            for p in range(NP):
                load((p + 2) // 2)
                mm_step(p)
                z_step(p + 1)
                inv_step(p - 1)
                ta_step(p)
                fwd_psT(p + 2)
                evac_store(p - 1)
            inv_step(NP - 1)
            evac_store(NP - 1)

    nc.finalize()
    _NC_CACHE["nc"] = nc
    return nc


def run(x, W, bias, trace=False):
    from concourse.bass_utils import run_bass_kernel_spmd
    import ml_dtypes

    x = np.ascontiguousarray(np.asarray(x, dtype=np.float32))
    W = np.asarray(W, dtype=np.float32)
    bias = np.asarray(bias, dtype=np.float32)
    assert x.shape == (8, 16, 512, 512), x.shape

    LT, c = _consts(W, bias)
    nc = _build_nc()
    identf = np.eye(128, dtype=np.float32)
    ident = identf.astype(ml_dtypes.bfloat16)
    LTh = LT.astype(ml_dtypes.bfloat16)
    in_maps = [
        {"xin": np.ascontiguousarray(x[i].reshape(8192, 512)), "ltw": LTh,
         "idw": ident, "idwf": identf}
        for i in range(_NCORES)
    ]
    res = run_bass_kernel_spmd(nc, in_maps, core_ids=list(range(_NCORES)),
                               trace=trace)
    out = np.stack(
        [res.results[i]["yout"].reshape(16, 512, 512) for i in range(_NCORES)]
    )
    if np.any(c):
        cimg = np.tile(c.reshape(8, 8), (64, 64)).astype(np.float32)
        out = out + cimg[None, None]
    return out.astype(np.float32), res


def kernel(x, W, bias):
    out, _ = run(x, W, bias, trace=False)
    return out


# revision 23
# speedup vs baseline: 2.1485x; 1.0069x over previous
"""BlockDCTSandwich Trainium2 kernel.

The whole op (blockify -> 8x8 DCT -> zigzag gather -> Linear(64,64) -> IDCT
-> deblockify) is a single fused 64x64 linear map per 8x8 block:
    out_vec = M @ x_vec + c,  M = kron(D^T,D^T) @ W @ G @ kron(D,D),
    c = kron(D^T,D^T) @ bias

Data-parallel over batch: one batch element (16 channels, 16.78 MB) per
NeuronCore. Work unit = a PAIR of [128, 512] image tiles (256 rows), software
pipelined so the DVE (the bottleneck engine, 2 stream-transpose passes over
all data) never stalls. Per pair (rows r = 8*hb + n, cols w = 8*wb + m,
wb = s*32 + sw, m = 2*cc + m0):

  load   natural rows (partition = r), SWDGE cast f32 -> bf16
  fwd    16 PE transposes (bf16, tile_position packs m0) -> psT PSUM bf16
            psT[m0*64+wb, cc*256 + t*128 + r] = x_t[r, wb*8 + cc*2 + m0]
  Z      1 DVE stream-transpose, 1024 cols (strided PSUM view) -> Z bf16
            Z[pi, (t*16+hb)*32 + sw] = x_t[hb*8+n, (s*32+sw)*8 + m]
            pi = m0*64 + s*32 + n*4 + cc
  mm     2 bf16 matmuls, stationary LT = M blockdiag over s -> ps PSUM f32
  T_a    1 DVE stream-transpose, 1024 cols              -> OYs SBUF f32
  inv    8 PE transposes (f32)                          -> psO PSUM f32
  evac   ACT copies (free-dim shuffle, cast) -> OXP bf16 -> store bf16
         (host upcasts the gathered output to f32)

PSUM budget: psT(1 bank) + ps(2) + psO(1 per tile) each double-buffered = 8.
Self-contained: hardcodes shapes x=(8,16,512,512) f32, W=(64,64), bias=(64,).
"""

import sys

import numpy as np

if "/opt/trn_rl_repo" not in sys.path:
    sys.path.insert(0, "/opt/trn_rl_repo")

_B = 8
_NCORES = 8


def _dct_matrix(b):
    n = np.arange(b)
    k = n[:, None]
    Dm = np.sqrt(2.0 / b) * np.cos(np.pi * (2 * n[None, :] + 1) * k / (2 * b))
    Dm[0] *= 1.0 / np.sqrt(2.0)
    return Dm


def _build_idx(b):
    def to_key(x):
        s = x[0] + x[1]
        o = b * b * s
        if s % 2 == 1:
            o += x[0]
        else:
            o -= x[0]
        return o

    coords = sorted(([i, j] for i in range(b) for j in range(b)), key=to_key)
    arr = np.array(coords).reshape(b, b, 2)
    return (np.arange(b)[None, :] * arr[..., 0] + arr[..., 1]).reshape(-1)


def _consts(W, bias):
    """Fused 64x64 map M as a 128x128 stationary lhsT (blockdiag over the
    column-half parity s), plus the bias image constant c.

    Partition encoding on both matmul sides: a = m0*64 + s*32 + n*4 + cc,
    with block-local coords (n, m), m = 2*cc + m0.
    """
    D = _dct_matrix(_B)
    idx = _build_idx(_B)
    G = np.zeros((64, 64))
    G[np.arange(64), idx] = 1.0
    M = np.kron(D.T, D.T) @ W.astype(np.float64) @ G @ np.kron(D, D)
    c = np.kron(D.T, D.T) @ bias.astype(np.float64)

    enc = np.arange(128)
    loc = 8 * ((enc >> 2) & 7) + 2 * (enc & 3) + (enc >> 6)
    spar = (enc >> 5) & 1
    LT = M[np.ix_(loc, loc)].T * (spar[:, None] == spar[None, :])
    return LT.astype(np.float32), c


_NC_CACHE = {}


def _build_nc():
    if "nc" in _NC_CACHE:
        return _NC_CACHE["nc"]
    import concourse.bass as bass
    import concourse.mybir as mybir
    from concourse import bacc
    from concourse.tile import TileContext

    f32 = mybir.dt.float32
    bf16 = mybir.dt.bfloat16
    ds = bass.ds

    nc = bacc.Bacc("TRN2", target_bir_lowering=False, debug=False,
                   num_devices=_NCORES)
    xin = nc.dram_tensor("xin", [8192, 512], f32, kind="ExternalInput")
    ltw = nc.dram_tensor("ltw", [128, 128], bf16, kind="ExternalInput")
    idw = nc.dram_tensor("idw", [128, 128], bf16, kind="ExternalInput")
    idwf = nc.dram_tensor("idwf", [128, 128], f32, kind="ExternalInput")
    yout = nc.dram_tensor("yout", [8192, 512], bf16, kind="ExternalOutput")

    xin_ap = xin.ap()
    yout_ap = yout.ap()

    with TileContext(nc) as tc:
        with (
            tc.tile_pool(name="wp", bufs=1) as wp,
            tc.tile_pool(name="io", bufs=4) as iop,
            tc.tile_pool(name="wk", bufs=6) as wk,
            tc.tile_pool(name="psp", bufs=2, space="PSUM") as psp,
        ):
            lt_sb = wp.tile([128, 128], bf16)
            id_sb = wp.tile([128, 128], bf16, tag="id_sb")
            idf_sb = wp.tile([128, 128], f32, tag="idf_sb")

            # Software-pipelined flat loop over 32 tile-PAIRS (each pair =
            # two [128, 512] tiles, 1024-wide DVE transposes to amortize the
            # PSUM-access init). Steady-state issue per iter p:
            #   mm_p | Z_{p+1} | inv_{p-1} | T_a_p | fwdT_{p+2} | evac/store
            # DVE (bottleneck) order [Z_{p+1}, T_a_p] never stalls: mm_p
            # (427ns) completes inside Z_{p+1} (1192ns).
            NP = 32
            XBs, psTs, Zs, pss, OYss, psOs = {}, {}, {}, {}, {}, {}

            def load(p):
                if p >= NP or p in XBs:
                    return
                XB = iop.tile([128, 1024], bf16, tag="XB")
                nc.gpsimd.dma_start(
                    out=XB[:, :].rearrange("p (t2 w) -> p t2 w", t2=2),
                    in_=xin_ap[ds(p * 256, 256), :]
                    .rearrange("(t2 r) w -> t2 r w", t2=2, r=128)
                    .transpose([1, 0, 2]),
                )
                XBs[p] = XB

            def fwd_psT(p):
                if p >= NP:
                    return
                load(p)
                psT = psp.tile([128, 1024], bf16, tag="psT")
                for t in range(2):
                    Xs = XBs.pop(p)[:, ds(t * 512, 512)] if t == 1 \
                        else XBs[p][:, ds(t * 512, 512)]
                    xv = Xs.rearrange("p (wb m) -> p wb m", wb=64, m=8)
                    for m in range(8):
                        cc, m0 = m >> 1, m & 1
                        nc.tensor.transpose(
                            psT[ds(64 * m0, 64),
                                ds(cc * 256 + t * 128, 128)],
                            in_=xv[:, :, m],
                            identity=id_sb[:, :],
                            tile_position=(0, 64 * m0),
                        )
                psTs[p] = psT

            def z_step(p, split=False):
                if p >= NP:
                    return
                Z = wk.tile([128, 1024], bf16, tag="Z")
                psT = psTs.pop(p)
                if split:  # two halves so DVE starts before full psT ready
                    for t in range(2):
                        nc.vector.transpose(
                            out=Z[:, ds(t * 512, 512)],
                            in_=psT[:, :].rearrange(
                                "p (cc t hb n) -> p t hb n cc",
                                cc=4, t=2, hb=16, n=8)[:, t],
                        )
                else:
                    nc.vector.transpose(
                        out=Z[:, :],
                        in_=psT[:, :].rearrange(
                            "p (cc thb n) -> p thb n cc", cc=4, thb=32, n=8),
                    )
                Zs[p] = Z

            def mm_step(p):
                ps = psp.tile([128, 1024], f32, tag="ps")
                Z = Zs.pop(p)
                for t in range(2):
                    nc.tensor.matmul(ps[:, ds(t * 512, 512)], lt_sb[:, :],
                                     Z[:, ds(t * 512, 512)],
                                     start=True, stop=True)
                pss[p] = ps

            def ta_step(p):
                OYs = wk.tile([128, 1024], f32, tag="OYs")
                nc.vector.transpose(
                    out=OYs[:, :].rearrange("p (cc thb n) -> p thb n cc",
                                            cc=4, thb=32, n=8),
                    in_=pss.pop(p)[:, :],
                )
                OYss[p] = OYs

            def inv_step(p):
                if p < 0:
                    return
                OYs = OYss.pop(p)
                tiles = []
                for t in range(2):
                    psO = psp.tile([128, 512], f32, tag="psO")
                    for cc in range(4):
                        nc.tensor.transpose(
                            psO[:, ds(cc * 128, 128)],
                            in_=OYs[:, ds(cc * 256 + t * 128, 128)],
                            identity=idf_sb[:, :],
                        )
                    tiles.append(psO)
                psOs[p] = tiles

            def evac_store(p):
                if p < 0:
                    return
                OXP = iop.tile([128, 1024], bf16, tag="OXP")
                for t, psO in enumerate(psOs.pop(p)):
                    nc.scalar.copy(
                        OXP[:, ds(t * 512, 512)].rearrange(
                            "p (sw cc m0) -> p cc m0 sw", sw=64, cc=4, m0=2),
                        psO[:, :].rearrange(
                            "p (cc m0 sw) -> p cc m0 sw", cc=4, m0=2, sw=64),
                    )
                    nc.sync.dma_start(
                        out=yout_ap[ds(p * 256 + t * 128, 128), :],
                        in_=OXP[:, ds(t * 512, 512)],
                    )

            # prologue: pair-0 loads as two single-tile DMAs (separate
            # tiles, so its first 8 PE transposes start after ~one 364ns
            # transfer); weight loads issue after the first data loads so
            # they don't delay them on the serial DMA engine pool
            XH0 = iop.tile([128, 512], bf16, tag="XH")
            nc.gpsimd.dma_start(out=XH0[:, :], in_=xin_ap[ds(0, 128), :])
            XH1 = iop.tile([128, 512], bf16, tag="XH")
            nc.gpsimd.dma_start(out=XH1[:, :], in_=xin_ap[ds(128, 128), :])
            nc.sync.dma_start(out=lt_sb[:, :], in_=ltw.ap())
            nc.sync.dma_start(out=id_sb[:, :], in_=idw.ap())
            nc.sync.dma_start(out=idf_sb[:, :], in_=idwf.ap())
            load(1)
            psT0 = psp.tile([128, 1024], bf16, tag="psT")
            for t, XH in enumerate((XH0, XH1)):
                xv = XH[:, :].rearrange("p (wb m) -> p wb m", wb=64, m=8)
                for m in range(8):
                    cc, m0 = m >> 1, m & 1
                    nc.tensor.transpose(
                        psT0[ds(64 * m0, 64), ds(cc * 256 + t * 128, 128)],
                        in_=xv[:, :, m],
                        identity=id_sb[:, :],
                        tile_position=(0, 64 * m0),
                    )
            psTs[0] = psT0
            z_step(0)
            fwd_psT(1)

            for p in range(NP):
                load(p + 2)
                mm_step(p)
                z_step(p + 1)
                inv_step(p - 1)
                ta_step(p)
                fwd_psT(p + 2)
                evac_store(p - 1)
            inv_step(NP - 1)
            evac_store(NP - 1)

    nc.finalize()
    _NC_CACHE["nc"] = nc
    return nc


def run(x, W, bias, trace=False):
    from concourse.bass_utils import run_bass_kernel_spmd
    import ml_dtypes

    x = np.ascontiguousarray(np.asarray(x, dtype=np.float32))
    W = np.asarray(W, dtype=np.float32)
    bias = np.asarray(bias, dtype=np.float32)
    assert x.shape == (8, 16, 512, 512), x.shape

    LT, c = _consts(W, bias)
    nc = _build_nc()
    identf = np.eye(128, dtype=np.float32)
    ident = identf.astype(ml_dtypes.bfloat16)
    LTh = LT.astype(ml_dtypes.bfloat16)
    in_maps = [
        {"xin": np.ascontiguousarray(x[i].reshape(8192, 512)), "ltw": LTh,
         "idw": ident, "idwf": identf}
        for i in range(_NCORES)
    ]
    res = run_bass_kernel_spmd(nc, in_maps, core_ids=list(range(_NCORES)),
                               trace=trace)
    out = np.stack(
        [np.asarray(res.results[i]["yout"], dtype=np.float32)
         .reshape(16, 512, 512) for i in range(_NCORES)]
    )
    if np.any(c):
        cimg = np.tile(c.reshape(8, 8), (64, 64)).astype(np.float32)
        out = out + cimg[None, None]
    return out.astype(np.float32), res


def kernel(x, W, bias):
    out, _ = run(x, W, bias, trace=False)
    return out


# revision 25
# speedup vs baseline: 2.1580x; 1.0044x over previous
"""BlockDCTSandwich Trainium2 kernel.

The whole op (blockify -> 8x8 DCT -> zigzag gather -> Linear(64,64) -> IDCT
-> deblockify) is a single fused 64x64 linear map per 8x8 block:
    out_vec = M @ x_vec + c,  M = kron(D^T,D^T) @ W @ G @ kron(D,D),
    c = kron(D^T,D^T) @ bias

Data-parallel over batch: one batch element (16 channels, 16.78 MB) per
NeuronCore. Work unit = a PAIR of [128, 512] image tiles (256 rows), software
pipelined so the DVE (the bottleneck engine, 2 stream-transpose passes over
all data) never stalls. Per pair (rows r = 8*hb + n, cols w = 8*wb + m,
wb = s*32 + sw, m = 2*cc + m0):

  load   natural rows (partition = r), SWDGE cast f32 -> bf16
  fwd    16 PE transposes (bf16, tile_position packs m0) -> psT PSUM bf16
            psT[m0*64+wb, cc*256 + t*128 + r] = x_t[r, wb*8 + cc*2 + m0]
  Z      1 DVE stream-transpose, 1024 cols (strided PSUM view) -> Z bf16
            Z[pi, (t*16+hb)*32 + sw] = x_t[hb*8+n, (s*32+sw)*8 + m]
            pi = m0*64 + s*32 + n*4 + cc
  mm     2 bf16 matmuls, stationary LT = M blockdiag over s -> ps PSUM f32
  T_a    1 DVE stream-transpose, 1024 cols              -> OYs SBUF f32
  inv    8 PE transposes (f32)                          -> psO PSUM f32
  evac   ACT copies (free-dim shuffle, cast) -> OXP bf16 -> store bf16
         (host upcasts the gathered output to f32)

PSUM budget: psT(1 bank) + ps(2) + psO(1 per tile) each double-buffered = 8.
Self-contained: hardcodes shapes x=(8,16,512,512) f32, W=(64,64), bias=(64,).
"""

import sys

import numpy as np

if "/opt/trn_rl_repo" not in sys.path:
    sys.path.insert(0, "/opt/trn_rl_repo")

_B = 8
_NCORES = 8


def _dct_matrix(b):
    n = np.arange(b)
    k = n[:, None]
    Dm = np.sqrt(2.0 / b) * np.cos(np.pi * (2 * n[None, :] + 1) * k / (2 * b))
    Dm[0] *= 1.0 / np.sqrt(2.0)
    return Dm


def _build_idx(b):
    def to_key(x):
        s = x[0] + x[1]
        o = b * b * s
        if s % 2 == 1:
            o += x[0]
        else:
            o -= x[0]
        return o

    coords = sorted(([i, j] for i in range(b) for j in range(b)), key=to_key)
    arr = np.array(coords).reshape(b, b, 2)
    return (np.arange(b)[None, :] * arr[..., 0] + arr[..., 1]).reshape(-1)


def _consts(W, bias):
    """Fused 64x64 map M as a 128x128 stationary lhsT (blockdiag over the
    column-half parity s), plus the bias image constant c.

    Partition encoding on both matmul sides: a = m0*64 + s*32 + n*4 + cc,
    with block-local coords (n, m), m = 2*cc + m0.
    """
    D = _dct_matrix(_B)
    idx = _build_idx(_B)
    G = np.zeros((64, 64))
    G[np.arange(64), idx] = 1.0
    M = np.kron(D.T, D.T) @ W.astype(np.float64) @ G @ np.kron(D, D)
    c = np.kron(D.T, D.T) @ bias.astype(np.float64)

    enc = np.arange(128)
    loc = 8 * ((enc >> 2) & 7) + 2 * (enc & 3) + (enc >> 6)
    spar = (enc >> 5) & 1
    LT = M[np.ix_(loc, loc)].T * (spar[:, None] == spar[None, :])
    return LT.astype(np.float32), c


_NC_CACHE = {}


def _build_nc():
    if "nc" in _NC_CACHE:
        return _NC_CACHE["nc"]
    import concourse.bass as bass
    import concourse.mybir as mybir
    from concourse import bacc
    from concourse.tile import TileContext

    f32 = mybir.dt.float32
    bf16 = mybir.dt.bfloat16
    ds = bass.ds

    nc = bacc.Bacc("TRN2", target_bir_lowering=False, debug=False,
                   num_devices=_NCORES)
    xin = nc.dram_tensor("xin", [8192, 512], f32, kind="ExternalInput")
    ltw = nc.dram_tensor("ltw", [128, 128], bf16, kind="ExternalInput")
    idw = nc.dram_tensor("idw", [128, 128], bf16, kind="ExternalInput")
    idwf = nc.dram_tensor("idwf", [128, 128], f32, kind="ExternalInput")
    yout = nc.dram_tensor("yout", [8192, 512], bf16, kind="ExternalOutput")

    xin_ap = xin.ap()
    yout_ap = yout.ap()

    with TileContext(nc) as tc:
        with (
            tc.tile_pool(name="wp", bufs=1) as wp,
            tc.tile_pool(name="io", bufs=4) as iop,
            tc.tile_pool(name="wk", bufs=6) as wk,
            tc.tile_pool(name="psp", bufs=2, space="PSUM") as psp,
        ):
            lt_sb = wp.tile([128, 128], bf16)
            nc.sync.dma_start(out=lt_sb[:, :], in_=ltw.ap())
            id_sb = wp.tile([128, 128], bf16, tag="id_sb")
            nc.sync.dma_start(out=id_sb[:, :], in_=idw.ap())
            idf_sb = wp.tile([128, 128], f32, tag="idf_sb")
            nc.sync.dma_start(out=idf_sb[:, :], in_=idwf.ap())

            # Software-pipelined flat loop over 32 tile-PAIRS (each pair =
            # two [128, 512] tiles, 1024-wide DVE transposes to amortize the
            # PSUM-access init). Steady-state issue per iter p:
            #   mm_p | Z_{p+1} | inv_{p-1} | T_a_p | fwdT_{p+2} | evac/store
            # DVE (bottleneck) order [Z_{p+1}, T_a_p] never stalls: mm_p
            # (427ns) completes inside Z_{p+1} (1192ns).
            NP = 32
            XBs, psTs, Zs, pss, OYss, psOs = {}, {}, {}, {}, {}, {}

            def load(p):
                if p >= NP or p in XBs:
                    return
                XB = iop.tile([128, 1024], bf16, tag="XB")
                nc.gpsimd.dma_start(
                    out=XB[:, :].rearrange("p (t2 w) -> p t2 w", t2=2),
                    in_=xin_ap[ds(p * 256, 256), :]
                    .rearrange("(t2 r) w -> t2 r w", t2=2, r=128)
                    .transpose([1, 0, 2]),
                )
                XBs[p] = XB

            def fwd_psT(p):
                if p >= NP:
                    return
                load(p)
                psT = psp.tile([128, 1024], bf16, tag="psT")
                for t in range(2):
                    Xs = XBs.pop(p)[:, ds(t * 512, 512)] if t == 1 \
                        else XBs[p][:, ds(t * 512, 512)]
                    xv = Xs.rearrange("p (wb m) -> p wb m", wb=64, m=8)
                    for m in range(8):
                        cc, m0 = m >> 1, m & 1
                        nc.tensor.transpose(
                            psT[ds(64 * m0, 64),
                                ds(cc * 256 + t * 128, 128)],
                            in_=xv[:, :, m],
                            identity=id_sb[:, :],
                            tile_position=(0, 64 * m0),
                        )
                psTs[p] = psT

            def z_step(p):
                if p >= NP:
                    return
                Z = wk.tile([128, 1024], bf16, tag="Z")
                nc.vector.transpose(
                    out=Z[:, :],
                    in_=psTs.pop(p)[:, :].rearrange(
                        "p (cc thb n) -> p thb n cc", cc=4, thb=32, n=8),
                )
                Zs[p] = Z

            def mm_step(p):
                ps = psp.tile([128, 1024], f32, tag="ps")
                Z = Zs.pop(p)
                for t in range(2):
                    nc.tensor.matmul(ps[:, ds(t * 512, 512)], lt_sb[:, :],
                                     Z[:, ds(t * 512, 512)],
                                     start=True, stop=True)
                pss[p] = ps

            def ta_step(p):
                OYs = wk.tile([128, 1024], f32, tag="OYs")
                nc.vector.transpose(
                    out=OYs[:, :].rearrange("p (cc thb n) -> p thb n cc",
                                            cc=4, thb=32, n=8),
                    in_=pss.pop(p)[:, :],
                )
                OYss[p] = OYs

            def inv_step(p):
                if p < 0:
                    return
                OYs = OYss.pop(p)
                tiles = []
                for t in range(2):
                    psO = psp.tile([128, 512], f32, tag="psO")
                    for cc in range(4):
                        nc.tensor.transpose(
                            psO[:, ds(cc * 128, 128)],
                            in_=OYs[:, ds(cc * 256 + t * 128, 128)],
                            identity=idf_sb[:, :],
                        )
                    tiles.append(psO)
                psOs[p] = tiles

            def evac_store(p):
                if p < 0:
                    return
                OXP = iop.tile([128, 1024], bf16, tag="OXP")
                for t, psO in enumerate(psOs.pop(p)):
                    nc.scalar.copy(
                        OXP[:, ds(t * 512, 512)].rearrange(
                            "p (sw cc m0) -> p cc m0 sw", sw=64, cc=4, m0=2),
                        psO[:, :].rearrange(
                            "p (cc m0 sw) -> p cc m0 sw", cc=4, m0=2, sw=64),
                    )
                    nc.sync.dma_start(
                        out=yout_ap[ds(p * 256 + t * 128, 128), :],
                        in_=OXP[:, ds(t * 512, 512)],
                    )

            # prologue
            load(0)
            load(1)
            fwd_psT(0)
            z_step(0)
            fwd_psT(1)

            for p in range(NP):
                load(p + 2)
                mm_step(p)
                z_step(p + 1)
                inv_step(p - 1)
                ta_step(p)
                fwd_psT(p + 2)
                evac_store(p - 1)
            inv_step(NP - 1)
            evac_store(NP - 1)

    nc.finalize()
    _NC_CACHE["nc"] = nc
    return nc


def run(x, W, bias, trace=False):
    from concourse.bass_utils import run_bass_kernel_spmd
    import ml_dtypes

    x = np.ascontiguousarray(np.asarray(x, dtype=np.float32))
    W = np.asarray(W, dtype=np.float32)
    bias = np.asarray(bias, dtype=np.float32)
    assert x.shape == (8, 16, 512, 512), x.shape

    LT, c = _consts(W, bias)
    nc = _build_nc()
    identf = np.eye(128, dtype=np.float32)
    ident = identf.astype(ml_dtypes.bfloat16)
    LTh = LT.astype(ml_dtypes.bfloat16)
    in_maps = [
        {"xin": np.ascontiguousarray(x[i].reshape(8192, 512)), "ltw": LTh,
         "idw": ident, "idwf": identf}
        for i in range(_NCORES)
    ]
    res = run_bass_kernel_spmd(nc, in_maps, core_ids=list(range(_NCORES)),
                               trace=trace)
    out = np.stack(
        [np.asarray(res.results[i]["yout"], dtype=np.float32)
         .reshape(16, 512, 512) for i in range(_NCORES)]
    )
    if np.any(c):
        cimg = np.tile(c.reshape(8, 8), (64, 64)).astype(np.float32)
        out = out + cimg[None, None]
    return out.astype(np.float32), res


def kernel(x, W, bias):
    out, _ = run(x, W, bias, trace=False)
    return out
